# revision 81
# baseline (speedup 1.0000x reference)
"""Trainium2 Bass kernel for nn_DARPDecoder (sparse_attention).

Strategy (pure data-parallel over batch, 8 cores x 128 batches):
  score[b,n] = emb[b,n,:] . qk[b] / sqrt(D) - travel[b,n]*c ; tanh-clip, mask,
  log_softmax.  qk[b] = W_key^T q[b] eliminates the [B,N,D] K intermediate.

Per core, two streaming passes over the embedding shard, both on ONE
prioritized sync-queue (order: tiny consts, fp8 nat stream, weight blob, bf16
transposed stream) so the sums pass is never bandwidth-starved while the
score stream still prefetches into an 11-deep pool:
  pass 1 (natural [n,d] tiles, FP8 -- the graph/visited sums are insensitive
    to fp8 quantization): accumulating matmuls with per-batch zero-padded
    [128,32] fp8 stationaries + tile_position, so each batch lands on its own
    PSUM rows.  Stationaries are memset/scattered on DVE+gpsimd; the inner
    loop is t-outer so chunk 0 only waits for the first stationary tile.
  pass 2 (host-pre-transposed [d,n] BF16 tiles; fp8 here fails the 2e-2
    gate): per-batch score matmuls with zero-padded qk columns accumulate
    into one [128,512] PSUM tile (batch -> partition).  The last chunk is
    split into 4 sub-DMAs to shorten the post-stream tail.
Travel lookup T[cur_h3[b], h3[b,n]]: indirect-DMA the 128 current rows (bf16),
then 16 gpsimd indirect_copy calls (8 batches each, host pre-wrapped indices)
gather along the free axis; DVE mask-accumulates the valid partitions into
-c*travel during pass 1, and one identity matmul per 32-row region folds it
into the score PSUM right after that region's start=True matmul.
h_current/h_first rows are indirect-DMA'd from a BF16 row-major copy that is
never streamed (only 2x32KB of it is ever read on-device).
Epilogue runs region-by-region as each 32-batch PSUM region completes
(after chunk 4J+3), overlapped with the remaining stream: tanh is computed
as 1-2/(e^{2s/C}+1) and softmax skips the max-subtraction (masked scores are
bounded by TANH_CLIP), so the whole program uses one activation-table set
(exp+ln) and no LoadActFuncSet lands on the critical tail.  Region outputs
DMA out on the gpsimd queue to avoid head-of-line-blocking the streams.
"""

import functools
import math

import numpy as np
import ml_dtypes

import concourse.bass as bass
import concourse.mybir as mybir
import concourse.tile as tile
from concourse import bacc
from concourse.bass_utils import run_bass_kernel_spmd

BF16 = mybir.dt.bfloat16
F8 = mybir.dt.float8e4
F32 = mybir.dt.float32
I32 = mybir.dt.int32
U16 = mybir.dt.uint16
U8 = mybir.dt.uint8
Alu = mybir.AluOpType
AF = mybir.ActivationFunctionType
AX = mybir.AxisListType

B, N, D, NCORES = 1024, 512, 128, 8
BC = B // NCORES  # 128 batches/core
NCH, CHB = 16, 8  # 16 stream chunks x 8 batches
MAX_TIME = 1440.0
TANH_CLIP = 10.0
C_TRAVEL = 1.0 / MAX_TIME / math.sqrt(2.0)
INV_SQRT_D = 1.0 / math.sqrt(D)
NBF = np.dtype(ml_dtypes.bfloat16)
NF8 = np.dtype(ml_dtypes.float8_e4m3)

# cbf blob column layout (bf16, [128, CBF_COLS])
_CB_W = 0          # w_last|w_first|w_graph|w_visited|w_keyT : 5*128
_CB_IDN = 640
_CB_PA = 768
_CB_PB = 896
_CB_WS = 1024      # rows 0:3 = W_state
CBF_COLS = 1152


def _emit(nc, tc, T):
    """Emit the whole per-core program. T: dict of dram tensor handles."""
    ap = {k: v.ap() for k, v in T.items()}

    with (
        tc.tile_pool(name="cp", bufs=1) as cp,
        tc.tile_pool(name="st", bufs=8) as st,
        tc.tile_pool(name="et", bufs=11) as et_pool,
        tc.tile_pool(name="tg", bufs=1) as tg,
        tc.tile_pool(name="wk", bufs=2) as wk,
        tc.tile_pool(name="ps_sum", bufs=1, space="PSUM") as ps_sum,
        tc.tile_pool(name="ps_sm", bufs=1, space="PSUM") as ps_sm,
        tc.tile_pool(name="ps_tr", bufs=2, space="PSUM") as ps_tr,
        tc.tile_pool(name="ps_sc", bufs=1, space="PSUM") as ps_sc,
    ):
        # ---------- packed const loads (sync queue; cbf+cu8 ride after nat) ----
        # current/previous/first node indices ride as exact f32 columns
        cf32 = cp.tile([128, 28], F32, name="cf32")
        nc.sync.dma_start(out=cf32[:], in_=ap["cf32"])
        h3w = cp.tile([128, 512], U16, name="h3w")
        nc.sync.dma_start(out=h3w[:], in_=ap["h3w"])
        visT8 = cp.tile([128, 512], U8, name="visT8")
        nc.sync.dma_start(out=visT8[:], in_=ap["visT8"])

        bst = cf32[:, 0:1]
        chf = cf32[:, 1:2]
        sc4 = cf32[:, 2:6]
        io5 = cf32[:, 6:7]
        curf = cf32[:, 24:25]
        prvf = cf32[:, 25:26]
        fstf = cf32[:, 26:27]

        # ---------- h3-row index for the ch3 gather (DVE, first) ----------
        gcf = cp.tile([BC, 1], F32, name="gcf")
        nc.vector.tensor_add(out=gcf[:], in0=io5, in1=curf)
        gcur = cp.tile([BC, 1], I32, name="gcur")
        nc.vector.tensor_copy(out=gcur[:], in_=gcf[:])

        # ---------- w2 stationaries, built on-device (DVE t=0..2, gpsimd t=3) --
        # per n-tile t: [128, 4096] fp8; batch b owns cols [32b, 32b+32):
        #   col 32b+2s = 1.0 (s = b%16) -> graph row; col 32b+2s+1 = vf_b.
        w2 = []
        for t in range(4):
            w = cp.tile([128, 4096], F8, name=f"w2_{t}", tag=f"w2_{t}")
            eng = nc.gpsimd if t >= 2 else nc.vector
            eng.memset(w[:], 0.0)
            ones_ap = w[:].rearrange("p (u c) -> p u c", u=8)[:, :, 0:512:34]
            eng.memset(ones_ap, 1.0)
            vf_ap = w[:].rearrange("p (u c) -> p u c", u=8)[:, :, 1:512:34]
            visT_t = visT8[:, 128 * t : 128 * (t + 1)]
            eng.tensor_copy(out=vf_ap, in_=visT_t.rearrange("p (u s) -> p u s", u=8))
            w2.append(w)

        # ---------- travel-chain gathers (Pool queue, ahead of hc/hf) ----------
        ch3 = cp.tile([BC, 1], I32, name="ch3")
        nc.gpsimd.indirect_dma_start(
            out=ch3[:], out_offset=None, in_=ap["h3_flat"],
            in_offset=bass.IndirectOffsetOnAxis(ap=gcur[:, :1], axis=0))
        rrow = cp.tile([BC, N], BF16, name="rrow")
        nc.gpsimd.indirect_dma_start(
            out=rrow[:], out_offset=None, in_=ap["ttm"],
            in_offset=bass.IndirectOffsetOnAxis(ap=ch3[:, :1], axis=0))
        # visited/action-mask bytes ride the gpsimd queue so the mask block can
        # run mid-pass-1 without waiting for the post-nat sync-queue slots
        cu8 = cp.tile([128, 1024], U8, name="cu8")
        nc.gpsimd.dma_start(out=cu8[:], in_=ap["cu8"])

        # ---------- first-node bookkeeping ----------
        t1 = cp.tile([BC, 1], F32, name="t1")
        nc.vector.tensor_single_scalar(out=t1[:], in_=prvf, scalar=0.0, op=Alu.is_equal)
        t2 = cp.tile([BC, 1], F32, name="t2")
        nc.vector.tensor_single_scalar(out=t2[:], in_=curf, scalar=0.0, op=Alu.not_equal)
        ld = cp.tile([BC, 1], F32, name="ld")
        nc.vector.tensor_mul(out=ld[:], in0=t1[:], in1=t2[:])
        dd = cp.tile([BC, 1], F32, name="dd")
        nc.vector.tensor_sub(out=dd[:], in0=curf, in1=fstf)
        nc.vector.tensor_mul(out=dd[:], in0=ld[:], in1=dd[:])
        fnf = cp.tile([BC, 1], F32, name="fnf")
        nc.vector.tensor_add(out=fnf[:], in0=fstf, in1=dd[:])
        nc.vector.tensor_mul(out=fnf[:], in0=fnf[:], in1=t2[:])
        gff = cp.tile([BC, 1], F32, name="gff")
        nc.vector.tensor_add(out=gff[:], in0=io5, in1=fnf[:])
        gfn = cp.tile([BC, 1], I32, name="gfn")
        nc.vector.tensor_copy(out=gfn[:], in_=gff[:])

        # ---------- hc/hf gathers (bf16 row-major copy, gather-only) ----------
        hc_rows = cp.tile([BC, D], BF16, name="hc_rows")
        nc.gpsimd.indirect_dma_start(
            out=hc_rows[:], out_offset=None, in_=ap["emb_flat"],
            in_offset=bass.IndirectOffsetOnAxis(ap=gcur[:, :1], axis=0))
        hf_rows = cp.tile([BC, D], BF16, name="hf_rows")
        nc.gpsimd.indirect_dma_start(
            out=hf_rows[:], out_offset=None, in_=ap["emb_flat"],
            in_offset=bass.IndirectOffsetOnAxis(ap=gfn[:, :1], axis=0))

        # ---------- travel gathers + combine ----------
        # call k serves batches {16g+k}; out[p,i] = rrow[p, h3[16g+k, i]].
        # The valid partitions of each call are selected and accumulated on
        # DVE (idle during pass 1) into trav [-c*travel], keeping PE free:
        # mask k (cf32 col 8+k) is -C_TRAVEL at partitions {16g+k}, else 0.
        qkp = cp.tile([128, 4096], BF16, name="qkp")
        nc.vector.memset(qkp[:], 0.0)
        trav = cp.tile([BC, N], F32, name="trav")
        tsel = cp.tile([BC, N], F32, name="tsel")

        def travel_combine(k):
            g = tg.tile([BC, N], BF16, tag=f"tg{k % 3}")
            nc.gpsimd.indirect_copy(out=g[:], data=rrow[:], idxs=h3w[:, 32 * k : 32 * k + 32],
                                    i_know_ap_gather_is_preferred=True)
            dst = trav if k == 0 else tsel
            nc.vector.tensor_scalar(out=dst[:], in0=g[:], scalar1=cf32[:, 8 + k : 9 + k],
                                    scalar2=None, op0=Alu.mult)
            if k > 0:
                nc.vector.tensor_add(out=trav[:], in0=trav[:], in1=tsel[:])

        for k in range(8):
            travel_combine(k)

        # ---------- masks / counts (interleaved so vcrb is ready before the
        # q-chain -- emitting them after the travel combine would stall the
        # vcrp matmul ~4us behind the in-order DVE stream) ----------
        visf = cp.tile([BC, N], F32, name="visf")
        nc.vector.tensor_copy(out=visf[:], in_=cu8[:, 0:512])
        amf = cp.tile([BC, N], F32, name="amf")
        nc.vector.tensor_copy(out=amf[:], in_=cu8[:, 512:1024])
        vc = cp.tile([BC, 1], F32, name="vc")
        nc.vector.tensor_reduce(out=vc[:], in_=visf[:], axis=AX.X, op=Alu.add)
        nc.vector.tensor_scalar_max(out=vc[:], in0=vc[:], scalar1=1.0)
        vcr = cp.tile([BC, 1], F32, name="vcr")
        nc.vector.reciprocal(out=vcr[:], in_=vc[:])
        vcrb = cp.tile([BC, 1], BF16, name="vcrb")
        nc.vector.tensor_copy(out=vcrb[:], in_=vcr[:])
        # tanh(s/C) = 1 - 2/(e^{2s/C}+1), so with r = 1/(e^{2s/C}+1):
        #   masked = C*am*tanh + (am-1)*1e8 = m12 - m20*r
        m10 = cp.tile([BC, N], F32, name="m10")
        nc.vector.tensor_scalar_mul(out=m10[:], in0=amf[:], scalar1=TANH_CLIP)
        m2 = cp.tile([BC, N], F32, name="m2")
        nc.vector.tensor_scalar(out=m2[:], in0=amf[:], scalar1=1.0, scalar2=1e8,
                                op0=Alu.subtract, op1=Alu.mult)
        m12 = cp.tile([BC, N], F32, name="m12")
        nc.vector.tensor_add(out=m12[:], in0=m10[:], in1=m2[:])
        m20 = cp.tile([BC, N], F32, name="m20")
        nc.vector.tensor_scalar_mul(out=m20[:], in0=m10[:], scalar1=2.0)

        for k in range(8, 16):
            travel_combine(k)
        travb = cp.tile([BC, N], BF16, name="travb")
        nc.vector.tensor_copy(out=travb[:], in_=trav[:])

        # ---------- pass 1: fp8 natural-layout stream -> graph/visited sums ----
        # w2 stationaries are host-packed (zeros/ones template + scattered vf):
        # per n-tile t, [128, 4096] fp8; batch b owns cols [32b, 32b+32):
        #   col 32b+2s = 1.0 (s = b%16) -> graph row; col 32b+2s+1 = vf_b.
        # single prioritized DMA queue: nat first, then cbf, then et --
        # two queues would fair-share bandwidth and delay the sums pass
        psA = ps_sum.tile([128, D], F32, tag="sumA")
        psB = ps_sum.tile([128, D], F32, tag="sumB")
        for k in range(NCH):
            nat = st.tile([128, 4096], F8, tag="nat")
            nc.sync.dma_start(out=nat[:], in_=ap["emb_nat_t"][k])
            for t in range(4):
                for j in range(CHB):
                    b = k * CHB + j
                    half, r = b // 64, b % 64
                    jj, s = r // 16, r % 16
                    ps = psA if half == 0 else psB
                    nc.tensor.matmul(
                        out=ps[32 * jj : 32 * jj + 32, :],
                        lhsT=w2[t][:, 32 * b : 32 * b + 32],
                        rhs=nat[:, (j * 4 + t) * 128 : (j * 4 + t + 1) * 128],
                        start=(s == 0 and t == 0), stop=True,
                        tile_position=(0, 32 * jj), skip_group_check=True)

        # ---------- weight/const blob (needed only from here on) ----------
        cbf = cp.tile([128, CBF_COLS], BF16, name="cbf")
        nc.sync.dma_start(out=cbf[:], in_=ap["cbf"])
        wl = cbf[:, _CB_W + 0 : _CB_W + 128]
        wf = cbf[:, _CB_W + 128 : _CB_W + 256]
        wg = cbf[:, _CB_W + 256 : _CB_W + 384]
        wv = cbf[:, _CB_W + 384 : _CB_W + 512]
        wkT = cbf[:, _CB_W + 512 : _CB_W + 640]
        idn = cbf[:, _CB_IDN : _CB_IDN + 128]
        pa = cbf[:, _CB_PA : _CB_PA + 128]
        pb = cbf[:, _CB_PB : _CB_PB + 128]
        ws = cbf[0:3, _CB_WS : _CB_WS + 128]

        # ---------- per-PSUM-row descale: even rows 1/512, odd rows 1/vcount ---
        vcrp = []
        for half, P in ((0, pa), (1, pb)):
            pm = ps_sm.tile([128, 1], F32, tag="sm")
            nc.tensor.matmul(out=pm[:], lhsT=P, rhs=vcrb[:], start=True, stop=True)
            vp = cp.tile([128, 1], F32, name=f"vcrp{half}", tag=f"vcrp{half}")
            nc.vector.tensor_add(out=vp[:], in0=pm[:], in1=chf)
            vcrp.append(vp)

        pssc = ps_sc.tile([128, N], F32, tag="score")

        # ---------- sums -> G^T / V^T (dense, bf16, [128e, 128b]) ----------
        gt = cp.tile([128, BC], BF16, name="gt")
        vt = cp.tile([128, BC], BF16, name="vt")
        for half, ps in ((0, psA), (1, psB)):
            gvr = wk.tile([128, 128], BF16, tag="gvr")
            nc.vector.tensor_scalar(out=gvr[:], in0=ps[:], scalar1=vcrp[half][:, :1],
                                    scalar2=None, op0=Alu.mult)
            pt = ps_tr.tile([128, 128], BF16, tag="gvt", bufs=1)
            nc.tensor.transpose(out=pt[:], in_=gvr[:], identity=idn)
            # cols m=32j+2s -> batch 64*half+16j+s
            src_g = pt[:].rearrange("p (j c) -> p j c", j=4)[:, :, 0:32:2]
            src_v = pt[:].rearrange("p (j c) -> p j c", j=4)[:, :, 1:32:2]
            dst_g = gt[:, 64 * half : 64 * half + 64].rearrange("p (j s) -> p j s", j=4)
            dst_v = vt[:, 64 * half : 64 * half + 64].rearrange("p (j s) -> p j s", j=4)
            nc.vector.tensor_copy(out=dst_g, in_=src_g)
            nc.vector.tensor_copy(out=dst_v, in_=src_v)

        # ---------- h_cur/h_first transposes ----------
        hct = cp.tile([128, BC], BF16, name="hct")
        pt1 = ps_tr.tile([128, 128], BF16, tag="gvt", bufs=1)
        nc.tensor.transpose(out=pt1[:], in_=hc_rows[:], identity=idn)
        nc.vector.tensor_copy(out=hct[:], in_=pt1[:])
        hft = cp.tile([128, BC], BF16, name="hft")
        pt2 = ps_tr.tile([128, 128], BF16, tag="gvt", bufs=1)
        nc.tensor.transpose(out=pt2[:], in_=hf_rows[:], identity=idn)
        nc.vector.tensor_copy(out=hft[:], in_=pt2[:])

        # ---------- state feats ----------
        sf = cp.tile([BC, 3], F32, name="sf")
        nc.vector.tensor_sub(out=sf[:, 0:1], in0=sc4[:, 2:3], in1=sc4[:, 1:2])
        nc.vector.tensor_scalar_mul(out=sf[:, 1:2], in0=sc4[:, 0:1], scalar1=1.0 / MAX_TIME)
        nc.vector.tensor_scalar_mul(out=sf[:, 2:3], in0=sc4[:, 3:4], scalar1=1.0 / (2.0 * N))
        sfb = cp.tile([BC, 3], BF16, name="sfb")
        nc.vector.tensor_copy(out=sfb[:], in_=sf[:])
        pt3 = ps_tr.tile([128, 128], BF16, tag="gvt", bufs=1)
        nc.tensor.transpose(out=pt3[:3, :], in_=sfb[:], identity=idn)
        sft = cp.tile([3, BC], BF16, name="sft")
        nc.vector.tensor_copy(out=sft[:], in_=pt3[:3, :BC])

        # ---------- q^T and qk^T ----------
        psq = ps_sm.tile([128, BC], F32, tag="sm")
        nc.tensor.matmul(out=psq[:], lhsT=wl, rhs=hct[:], start=True, stop=True)
        nc.tensor.matmul(out=psq[:], lhsT=wf, rhs=hft[:], start=False, stop=True,
                         skip_group_check=True)
        nc.tensor.matmul(out=psq[:], lhsT=wg, rhs=gt[:], start=False, stop=True,
                         skip_group_check=True)
        nc.tensor.matmul(out=psq[:], lhsT=wv, rhs=vt[:], start=False, stop=True,
                         skip_group_check=True)
        nc.tensor.matmul(out=psq[:], lhsT=ws, rhs=sft[:], start=False, stop=True,
                         skip_group_check=True)
        qt = cp.tile([128, BC], BF16, name="qt")
        nc.vector.tensor_scalar(out=qt[:], in0=psq[:], scalar1=bst, scalar2=None,
                                op0=Alu.add)
        psk = ps_sm.tile([128, BC], F32, tag="sm")
        nc.tensor.matmul(out=psk[:], lhsT=wkT, rhs=qt[:], start=True, stop=True)
        qk = cp.tile([128, BC], BF16, name="qk")
        nc.vector.tensor_scalar_mul(out=qk[:], in0=psk[:], scalar1=INV_SQRT_D)

        # ---------- qkpad: batch b -> col 32b + (b%32) (tile memset earlier) ---
        for J in range(4):
            # batch b = 32J + r -> col 32b + r = 1024J + 33r (out row = 32J + r = b)
            nc.vector.tensor_copy(out=qkp[:, 1024 * J : 1024 * (J + 1) : 33],
                                  in_=qk[:, 32 * J : 32 * (J + 1)])

        # ---------- pass 2: transposed stream -> score psum [128b, 512n] ----------
        # PSUM region J (batches 32J..32J+32) is final after chunk 4J+3, so the
        # full epilogue (incl. log-softmax and the output DMA) runs region-by-
        # region, overlapped with the remaining chunks.  tanh is computed via
        # exp so the whole program needs one act-table set (exp+ln): no
        # LoadActFuncSet ever lands on the critical tail.
        eu = cp.tile([BC, N], F32, name="eu")
        er = cp.tile([BC, N], F32, name="er")
        msk = cp.tile([BC, N], F32, name="msk")
        ex = cp.tile([BC, N], F32, name="ex")
        se = cp.tile([BC, 1], F32, name="se")
        lse = cp.tile([BC, 1], F32, name="lse")
        fin = cp.tile([BC, N], F32, name="fin")

        sea = cp.tile([BC, 1], F32, name="sea")

        def epilogue_region(J, split=False):
            r = slice(32 * J, 32 * J + 32)
            # masked = m12 - m20 / (exp(2s/C) + 1); since masked <= TANH_CLIP
            # the exp cannot overflow, so no max-subtraction pass is needed.
            # split=True pipelines column halves across ACT and DVE -- used for
            # the final region, whose chain is the post-stream critical tail.
            halves = [slice(0, N // 2), slice(N // 2, N)] if split else [slice(0, N)]
            for h in halves:
                nc.scalar.activation(out=eu[r, h], in_=pssc[r, h], func=AF.Exp,
                                     scale=2.0 / TANH_CLIP)
            for i, h in enumerate(halves):
                nc.vector.tensor_single_scalar(out=eu[r, h], in_=eu[r, h], scalar=1.0,
                                               op=Alu.add)
                nc.vector.reciprocal(out=er[r, h], in_=eu[r, h])
                nc.vector.tensor_mul(out=er[r, h], in0=er[r, h], in1=m20[r, h])
                nc.vector.tensor_sub(out=msk[r, h], in0=m12[r, h], in1=er[r, h])
                acc = se if i == 0 else sea
                nc.scalar.activation(out=ex[r, h], in_=msk[r, h], func=AF.Exp,
                                     scale=1.0, accum_out=acc[r, :])
            if split:
                nc.vector.tensor_add(out=se[r, :], in0=se[r, :], in1=sea[r, :])
            nc.scalar.activation(out=lse[r, :], in_=se[r, :], func=AF.Ln)
            nc.vector.tensor_scalar(out=fin[r, :], in0=msk[r, :], scalar1=lse[r, :1],
                                    scalar2=None, op0=Alu.subtract)
            # per-region output DMA on the idle gpsimd queue: a slot in either
            # stream queue would head-of-line-block the chunk loads behind it.
            # The final region rides the sync queue instead -- it is empty by
            # then and its descriptor path is faster than gpsimd's.
            if split:
                nc.sync.dma_start(out=ap["out"][r, :], in_=fin[r, :])
            else:
                nc.gpsimd.dma_start(out=ap["out"][r, :], in_=fin[r, :])

        for k in range(NCH):
            et = et_pool.tile([128, 4096], BF16, tag="et")
            if k == NCH - 1:
                # split the final chunk so its last matmuls wait on 2 batches
                # of bytes, not 8 -- trims the post-stream tail
                for q in range(4):
                    nc.sync.dma_start(out=et[:, 1024 * q : 1024 * (q + 1)],
                                      in_=ap["emb_T_t"][k][:, 1024 * q : 1024 * (q + 1)])
            else:
                nc.sync.dma_start(out=et[:], in_=ap["emb_T_t"][k])
            for j in range(CHB):
                b = k * CHB + j
                J = b // 32
                nc.tensor.matmul(
                    out=pssc[32 * J : 32 * J + 32, :],
                    lhsT=qkp[:, 32 * b : 32 * b + 32],
                    rhs=et[:, j * N : (j + 1) * N],
                    start=(b % 32 == 0), stop=True,
                    tile_position=(0, 32 * J), skip_group_check=True)
            if k % 4 == 0:
                # fold -c*travel into this region's freshly-started PSUM rows
                J = k // 4
                nc.tensor.matmul(
                    out=pssc[32 * J : 32 * J + 32, :],
                    lhsT=idn[:, 32 * J : 32 * J + 32], rhs=travb[:],
                    start=False, stop=True,
                    tile_position=(0, 32 * J), skip_group_check=True)
            if k % 4 == 3:
                epilogue_region(k // 4, split=(k == NCH - 1))


def build_program():
    nc = bacc.Bacc("TRN2", target_bir_lowering=False, debug=False)
    dt = nc.dram_tensor
    T = {}

    def din(name, shape, dtype):
        T[name] = dt(name, shape, dtype, kind="ExternalInput")

    din("emb_nat_t", [NCH, 128, CHB * N * D // 128], F8)
    din("emb_T_t", [NCH, 128, CHB * N], BF16)
    din("emb_flat", [BC * N, D], BF16)
    din("h3_flat", [BC * N, 1], I32)
    din("ttm", [N, N], BF16)
    din("cbf", [128, CBF_COLS], BF16)
    din("visT8", [128, 512], U8)
    din("cf32", [128, 28], F32)
    din("cu8", [128, 1024], U8)
    din("h3w", [128, 512], U16)
    T["out"] = dt("out", [BC, N], F32, kind="ExternalOutput")

    with tile.TileContext(nc) as tc:
        _emit(nc, tc, T)
    nc.compile()
    return nc


@functools.cache
def _cached_program():
    return build_program()


@functools.cache
def _consts():
    bidx = np.arange(BC)
    c = {}
    c["ident"] = np.eye(128, dtype=NBF)
    p = np.arange(128)
    c["tmask"] = np.where(p[:, None] % 16 == np.arange(16)[None, :],
                          np.float32(-C_TRAVEL), np.float32(0.0)).astype(np.float32)
    pa = np.zeros((128, 128), dtype=NBF)
    pb = np.zeros((128, 128), dtype=NBF)
    for b in range(64):
        m = 32 * (b // 16) + 2 * (b % 16) + 1
        pa[b, m] = 1
        pb[64 + b, m] = 1
    c["p_a"], c["p_b"] = pa, pb
    ch = np.zeros((128, 1), np.float32)
    ch[0::2] = 1.0 / N
    c["c_half"] = ch
    c["io5"] = (bidx.astype(np.float32) * N)[:, None]
    return c


def make_in_map(inputs, core, consts=None):
    """Host-side shard + relayout for one core (pure layout/dtype work)."""
    c = consts or _consts()
    sl = slice(BC * core, BC * (core + 1))
    emb = np.asarray(inputs["node_emb"][sl], dtype=np.float32)
    embb = emb.astype(NBF)  # [128, 512, 128]
    emb8 = emb.astype(NF8)
    m = {}
    m["emb_nat_t"] = np.ascontiguousarray(
        emb8.reshape(NCH, CHB, 4, 128, D).transpose(0, 3, 1, 2, 4)).reshape(NCH, 128, CHB * 4 * D)
    embT = np.ascontiguousarray(embb.transpose(0, 2, 1))  # [128b, 128d, 512n]
    m["emb_T_t"] = np.ascontiguousarray(
        embT.reshape(NCH, CHB, 128, N).transpose(0, 2, 1, 3)).reshape(NCH, 128, CHB * N)
    m["emb_flat"] = embb.reshape(BC * N, D)
    h3 = np.asarray(inputs["h3_indices"][sl]).astype(np.int32)  # [128, 512]
    m["h3_flat"] = h3.reshape(BC * N, 1)
    h3wrap = np.ascontiguousarray(
        h3.reshape(8, 16, 32, 16).transpose(1, 0, 3, 2)).reshape(16, 128, 32).astype(np.uint16)
    m["h3w"] = np.ascontiguousarray(h3wrap.transpose(1, 0, 2)).reshape(128, 512)
    m["ttm"] = np.asarray(inputs["travel_time_matrix"], np.float32).astype(NBF)
    vis = np.asarray(inputs["visited"][sl]).astype(np.uint8)
    m["cu8"] = np.ascontiguousarray(np.concatenate(
        [vis, np.asarray(inputs["action_mask"][sl]).astype(np.uint8)], axis=1))
    m["visT8"] = np.ascontiguousarray(np.concatenate(
        [vis[:, 128 * t : 128 * (t + 1)].T for t in range(4)], axis=1))

    ws_pad = np.zeros((128, 128), dtype=NBF)
    ws_pad[0:3, :] = np.asarray(inputs["W_state"], np.float32).astype(NBF)
    m["cbf"] = np.ascontiguousarray(np.concatenate(
        [np.asarray(inputs["W_last"], np.float32).astype(NBF),
         np.asarray(inputs["W_first"], np.float32).astype(NBF),
         np.asarray(inputs["W_graph"], np.float32).astype(NBF),
         np.asarray(inputs["W_visited"], np.float32).astype(NBF),
         np.ascontiguousarray(np.asarray(inputs["W_key"], np.float32).T).astype(NBF),
         c["ident"], c["p_a"], c["p_b"], ws_pad], axis=1))

    cf32 = np.zeros((128, 28), np.float32)
    cf32[:, 0] = np.asarray(inputs["b_state"], np.float32)
    cf32[:, 1:2] = c["c_half"]
    cf32[:, 2] = np.asarray(inputs["current_time"][sl], np.float32)[:, 0]
    cf32[:, 3] = np.asarray(inputs["used_capacity"][sl], np.float32)[:, 0]
    cf32[:, 4] = np.asarray(inputs["vehicle_capacity"][sl], np.float32)[:, 0]
    cf32[:, 5] = np.asarray(inputs["i"][sl]).astype(np.float32)[:, 0]
    cf32[:, 6:7] = c["io5"]
    cf32[:, 8:24] = c["tmask"]
    cf32[:, 24] = np.asarray(inputs["current_node"][sl]).astype(np.float32)[:, 0]
    cf32[:, 25] = np.asarray(inputs["previous_action"][sl]).astype(np.float32)[:, 0]
    cf32[:, 26] = np.asarray(inputs["first_node"][sl]).astype(np.float32)
    m["cf32"] = cf32
    return m


_last_results = None


def kernel(**inputs):
    global _last_results
    nc = _cached_program()
    consts = _consts()
    in_maps = [make_in_map(inputs, c, consts) for c in range(NCORES)]
    import os
    trace = bool(int(os.environ.get("KERNEL_TRACE", "0")))
    rr = run_bass_kernel_spmd(nc, in_maps, list(range(NCORES)), trace=trace)
    _last_results = rr
    out = np.concatenate([np.asarray(rr.results[c]["out"], np.float32)
                          for c in range(NCORES)], axis=0)
    return out
